# revision 1
# baseline (speedup 1.0000x reference)
"""BitConv1d Trainium2 kernel (8 NeuronCores, data-parallel over batch).

Reference semantics (per batch b):
    x_n   = rmsnorm_over_C(x) * gamma
    scale = max(|x_n|) over the WHOLE tensor (global -> AllGather + max)
    n     = round(clip(x_n / scale * 127, -128, 127))        (integers in [-127,127])
    w_s   = max(mean(|w|), 1e-4)
    w_q   = round(clip(w / w_s, -1, 1))                      (ternary)
    out   = conv1d(n, w_q, pad=3) * (scale/127) * w_s

Key insight: n is an integer |n|<=127 (exact in bf16) and w_q is ternary
(exact in bf16), so the conv is EXACT integer arithmetic on the PE in
bf16 with fp32 PSUM accumulation.  All rounding is done with the fp32
magic-number trick (+1.5*2^23, RNE) which matches jnp.round.

Per core: batch b = core_id, x slice [512, 8192].
  The host pre-permutes the weight to [cin, k, cout] so the quantized
    lhsT tiles are contiguous slices (no on-chip transposes).
  Phase A (per 512-col chunk): one 3-D DMA loads all 4 channel tiles;
    sum_c x^2 via 4 accumulating all-ones fp16 matmuls (partition-reduce
    AND broadcast in one); rms via the ACT rsqrt table (max rel err
    ~4e-5 measured); normalize on DVE; one abs-max reduce per chunk;
    one 3-D DMA stores x_n to DRAM scratch.  Weight quantization
    (|w|-mean, round, clip) overlaps the stream.
  AllGather of the 1-scalar local max + local reduce -> global scale.
  Phase B: chunks grouped (6/6/4); for each stationary weight tile the
    6 chunks' matmuls run back-to-back; 112 matmuls of [128x128]@[128x512]
    per chunk accumulate in PSUM; ACT scales by (w_s*scale/127); DMA out.
  Even/odd-shifted bf16 copies of the quantized activations keep every
    matmul rhs slice 4-byte aligned (odd offsets fault the PE).
"""

import os
import sys
import types

import numpy as np


def _install_ntff_shim():
    """Make bass_utils' trace path work in containers lacking antenv.axon_hooks."""
    try:
        import antenv.axon_hooks  # noqa: F401
        return
    except ImportError:
        pass
    try:
        from trn_agent_boot.trn_boot import _ntff_profile_via_ctypes

        mod = types.ModuleType("antenv.axon_hooks")
        hook = _ntff_profile_via_ctypes("/opt/axon/libaxon_pjrt.so")
        mod.get_axon_ntff_profile_hook = lambda: hook
        mod.set_axon_ntff_profile_hook = lambda h: None
        sys.modules["antenv.axon_hooks"] = mod
        import antenv

        antenv.axon_hooks = mod
    except Exception:
        pass


_install_ntff_shim()


def _install_ldw_opt_patch():
    """walrus emits one LDWEIGHTS per matmul unless ldw-opt dedupes
    consecutive loads of the same stationary weights; bass hardcodes the
    flag off, so rewrite it on the way to the driver."""
    if os.environ.get("BITCONV_LDWOPT", "0") != "1":
        return
    from concourse import bass_utils as _bu

    if getattr(_bu, "_bitconv_ldw_patched", False):
        return
    _orig = _bu.run_command

    def _patched(cmd, **kw):
        cmd = ["--enable-ldw-opt=true" if c == "--enable-ldw-opt=false" else c
               for c in cmd]
        return _orig(cmd, **kw)

    _bu.run_command = _patched
    _bu._bitconv_ldw_patched = True


_install_ldw_opt_patch()

import concourse.bacc as bacc
import concourse.tile as tile
from concourse import mybir
from concourse.bass_utils import run_bass_kernel_spmd
from concourse.masks import make_identity

f32 = mybir.dt.float32
bf16 = mybir.dt.bfloat16

N_CORES = 8
C = 512          # in/out channels
T = 8192         # sequence length
KS = 7           # kernel taps
PAD = 3
NT = 4           # channel tiles of 128
CH = 512         # T-chunk width
NCH = T // CH    # 16
EPS = 1e-6
QP = 127.0
MAGIC = 12582912.0        # 1.5 * 2**23 : fp32 round-to-nearest-int magic
W_ELEMS = C * C * KS      # 1835008
HALO = CH + 2 * PAD       # 518
GROUPS = [list(range(0, 6)), list(range(6, 12)), list(range(12, 16))]


def _build(apply_gamma: bool):
    Alu = mybir.AluOpType
    ACTF = mybir.ActivationFunctionType

    _exact_rms = os.environ.get("BITCONV_EXACT_RMS", "0") == "1"
    nc = bacc.Bacc("TRN2", target_bir_lowering=False, debug=False,
                   num_devices=N_CORES)

    x_ext = nc.dram_tensor("x", [C, T], f32, kind="ExternalInput")
    # host supplies weight transposed to [cin, k, cout] so quantized lhsT
    # tiles are contiguous slices (no on-chip transposes needed)
    w_ext = nc.dram_tensor("w", [C, KS, C], f32, kind="ExternalInput")
    nw_ext = nc.dram_tensor("nw", [C], f32, kind="ExternalInput")
    out_ext = nc.dram_tensor("out", [C, T], f32, kind="ExternalOutput")

    with tile.TileContext(nc) as tc:
        with (
            tc.tile_pool(name="consts", bufs=1) as consts,
            tc.tile_pool(name="wqt", bufs=1) as wqtp,
            tc.tile_pool(name="dram", bufs=1, space="DRAM") as dram,
        ):
            ones128 = consts.tile([128, 128], f32)
            nc.vector.memset(ones128[:], 1.0)
            ones_h = consts.tile([128, 128], mybir.dt.float16)
            nc.vector.memset(ones_h[:], 1.0)
            ident = consts.tile([128, 128], f32)
            make_identity(nc, ident[:])
            eps_t = consts.tile([128, 1], f32)
            nc.vector.memset(eps_t[:], EPS)
            gamma = [consts.tile([128, 1], f32, name=f"gamma{j}") for j in range(NT)]
            for j in range(NT):
                nc.sync.dma_start(
                    out=gamma[j][:],
                    in_=nw_ext[j * 128:(j + 1) * 128].rearrange("(p o) -> p o", o=1),
                )
            mxbuf = consts.tile([128, NCH], f32)        # abs-max per chunk
            wsums = consts.tile([128, NT], f32)
            # post-collective scalars
            sc128 = consts.tile([128, 1], f32)      # global act scale
            s127 = consts.tile([128, 1], f32)       # 127/scale
            gs = [consts.tile([128, 1], f32, name=f"gs{j}") for j in range(NT)]
            ws128 = consts.tile([128, 1], f32)      # weight scale
            osc = consts.tile([128, 1], f32)        # w_s*scale/127

            # ternary weights, bf16, lhsT layout: tile j holds
            # [128 cin, (k, cout)] so slice (k, m) is contiguous
            wqTs = [wqtp.tile([128, KS * C], bf16, name=f"wqT{j}")
                    for j in range(NT)]

            def wqT_sl(k, j, m):
                return wqTs[j][:, k * C + m * 128: k * C + m * 128 + 128]

            xn_scr = dram.tile([NT, 128, T], f32)
            ccin = dram.tile([1, 1], f32)
            _use_ag = os.environ.get("BITCONV_AG", "1") == "1"
            if _use_ag:
                ccag = dram.tile([N_CORES, 1], f32, addr_space="Shared")
            else:
                ccag = dram.tile([1, 1], f32)

            with (
                tc.tile_pool(name="xin", bufs=4) as xinp,
                tc.tile_pool(name="sq", bufs=3) as sqp,
                tc.tile_pool(name="rms", bufs=3) as rmsp,
                tc.tile_pool(name="xna", bufs=4) as xnap,
                tc.tile_pool(name="wraw", bufs=4) as wrawp,
                tc.tile_pool(name="wsm", bufs=2) as wsmp,
                tc.tile_pool(name="psA", bufs=3, space="PSUM") as psA,
                tc.tile_pool(name="psT", bufs=2, space="PSUM") as psT,
                tc.tile_pool(name="psW", bufs=1, space="PSUM") as psW,
                tc.tile_pool(name="smal", bufs=2) as smal,
            ):
                # ---- weight load + |w| sums (traced first; DMA overlaps) ---
                wraws = []
                for m in range(NT):
                    wraw = wrawp.tile([128, KS * C], f32)
                    nc.sync.dma_start(
                        out=wraw[:],
                        in_=w_ext[m * 128:(m + 1) * 128, :, :].rearrange(
                            "p k c -> p (k c)"))
                    wraws.append(wraw)
                    t56 = wsmp.tile([128, 56], f32)
                    nc.vector.tensor_reduce(
                        out=t56[:],
                        in_=wraw[:].rearrange("p (a b) -> p a b", b=64),
                        axis=mybir.AxisListType.X, op=Alu.add,
                        apply_absolute_value=True)
                    nc.vector.tensor_reduce(
                        out=wsums[:, m:m + 1], in_=t56[:],
                        axis=mybir.AxisListType.X, op=Alu.add)

                def w_scale_setup():
                    wtot = wsmp.tile([128, 1], f32)
                    nc.vector.tensor_reduce(out=wtot[:], in_=wsums[:],
                                            axis=mybir.AxisListType.X,
                                            op=Alu.add)
                    pws = psW.tile([128, 1], f32)
                    nc.tensor.matmul(pws[:], ones128[:], wtot[:],
                                     start=True, stop=True)
                    wmean = wsmp.tile([128, 1], f32)
                    nc.scalar.activation(out=wmean[:], in_=pws[:],
                                         func=ACTF.Copy, scale=1.0 / W_ELEMS)
                    nc.vector.tensor_scalar_max(ws128[:], wmean[:], 1e-4)
                    winv = wsmp.tile([128, 1], f32)
                    nc.vector.reciprocal(winv[:], ws128[:])
                    for m in range(NT):
                        # in-place: wraw <- round(w/ws)+MAGIC, clip to MAGIC+-1
                        nc.scalar.activation(out=wraws[m][:], in_=wraws[m][:],
                                             func=ACTF.Copy, scale=winv[:],
                                             bias=MAGIC)
                        nc.gpsimd.tensor_scalar(out=wraws[m][:],
                                                in0=wraws[m][:],
                                                scalar1=MAGIC + 1.0,
                                                scalar2=MAGIC - 1.0,
                                                op0=Alu.min, op1=Alu.max)

                def w_convert(j, half):
                    # -MAGIC and cast: ternary values, exact in bf16
                    h = (KS * C) // 2
                    nc.scalar.activation(
                        out=wqTs[j][:, half * h:(half + 1) * h],
                        in_=wraws[j][:, half * h:(half + 1) * h],
                        func=ACTF.Copy, scale=1.0, bias=-MAGIC)

                # ---- phase A: rmsnorm + local max, stream x_n to scratch ----
                # weight quant/transposes interleave with chunks 1..8 so the
                # PE FIFO stays clear for phase A's accumulation matmuls
                for ti in range(NCH):
                    if ti == 1:
                        w_scale_setup()
                    t0 = ti * CH
                    ps = psA.tile([128, CH], f32)
                    # one 3-D DMA brings all four channel tiles of the chunk
                    xt = xinp.tile([128, NT, CH], f32)
                    nc.sync.dma_start(
                        out=xt[:],
                        in_=x_ext[:, t0:t0 + CH].rearrange(
                            "(j p) t -> p j t", p=128))
                    sq = sqp.tile([128, NT, CH], mybir.dt.float16)
                    nc.scalar.square(sq[:], xt[:])
                    for j in range(NT):
                        # accumulate sum_c x^2 on the PE; all-ones lhsT also
                        # broadcasts the result to every partition
                        nc.tensor.matmul(ps[:], ones_h[:], sq[:, j, :],
                                         start=(j == 0), stop=(j == NT - 1))
                    if 2 <= ti <= 5:
                        w_convert(ti - 2, 0)
                        w_convert(ti - 2, 1)
                    rms = rmsp.tile([128, CH], f32)
                    if _exact_rms:
                        sqv = rmsp.tile([128, CH], f32)
                        nc.scalar.activation(out=sqv[:], in_=ps[:],
                                             func=ACTF.Sqrt,
                                             bias=eps_t[:], scale=1.0 / C)
                        nc.vector.reciprocal(rms[:], sqv[:])
                    else:
                        # table rsqrt (max rel err ~4e-5, measured) saves the
                        # 3.3us DVE reciprocal on the per-chunk critical path
                        nc.scalar.activation(out=rms[:], in_=ps[:],
                                             func=ACTF.Abs_reciprocal_sqrt,
                                             bias=eps_t[:], scale=1.0 / C)
                    xna = xnap.tile([128, NT, CH], f32)
                    for j in range(NT):
                        xsl = xna[:, j, :]
                        eng = nc.vector
                        if apply_gamma:
                            # x*gamma first (exact when gamma==1 anyway)
                            eng.tensor_scalar_mul(xsl, xt[:, j, :], gamma[j][:])
                            eng.tensor_mul(xsl, xsl, rms[:])
                        else:
                            eng.tensor_mul(xsl, xt[:, j, :], rms[:])
                    # one 3-D DMA stores the whole normalized chunk
                    nc.sync.dma_start(
                        out=xn_scr[:, :, t0:t0 + CH].rearrange("j p t -> p j t"),
                        in_=xna[:])
                    # one abs-max reduce per chunk covers all four tiles
                    nc.vector.tensor_reduce(
                        out=mxbuf[:, ti:ti + 1], in_=xna[:],
                        axis=mybir.AxisListType.XY, op=Alu.max,
                        apply_absolute_value=True)

                # ---- local max tree + AllGather(max) ----
                mx1 = smal.tile([128, 1], f32)
                nc.vector.tensor_reduce(out=mx1[:], in_=mxbuf[:],
                                        axis=mybir.AxisListType.X, op=Alu.max)
                mxt = smal.tile([1, 128], f32)
                nc.sync.dma_start(out=mxt[:], in_=mx1[:])
                mxs = smal.tile([1, 1], f32)
                nc.vector.tensor_reduce(out=mxs[:], in_=mxt[:],
                                        axis=mybir.AxisListType.X, op=Alu.max)
                mxc = smal.tile([1, 1], f32)
                nc.vector.tensor_scalar_max(mxc[:], mxs[:], 1e-5)
                nc.gpsimd.dma_start(out=ccin[:], in_=mxc[:])
                if _use_ag:
                    nc.gpsimd.collective_compute(
                        "AllGather", Alu.bypass,
                        replica_groups=[list(range(N_CORES))],
                        ins=[ccin.opt()], outs=[ccag.opt()],
                    )
                else:
                    nc.gpsimd.collective_compute(
                        "AllReduce", Alu.max,
                        replica_groups=[list(range(N_CORES))],
                        ins=[ccin.opt()], outs=[ccag.opt()],
                    )

                # ---- post-collective scalar setup ----
                if _use_ag:
                    agt = smal.tile([1, N_CORES], f32)
                    nc.gpsimd.dma_start(out=agt[:],
                                        in_=ccag[:].rearrange("r o -> o r"))
                    scs = smal.tile([1, 1], f32)
                    nc.vector.tensor_reduce(out=scs[:], in_=agt[:],
                                            axis=mybir.AxisListType.X, op=Alu.max)
                else:
                    scs = smal.tile([1, 1], f32)
                    nc.gpsimd.dma_start(out=scs[:], in_=ccag[:])
                nc.gpsimd.partition_broadcast(sc128[:], scs[:])
                sinv = smal.tile([128, 1], f32)
                nc.vector.reciprocal(sinv[:], sc128[:])
                nc.vector.tensor_scalar_mul(s127[:], sinv[:], QP)
                for j in range(NT):
                    if apply_gamma:
                        nc.vector.tensor_mul(gs[j][:], gamma[j][:], s127[:])
                    else:
                        nc.vector.tensor_copy(out=gs[j][:], in_=s127[:])
                nc.vector.tensor_mul(osc[:], ws128[:], sc128[:])
                nc.vector.tensor_scalar_mul(osc[:], osc[:], 1.0 / QP)

            # ---------------- Phase B: quantize + conv matmuls ---------------
            with (
                tc.tile_pool(name="xni", bufs=2) as xnip,
                tc.tile_pool(name="qf", bufs=2) as qfp,
                tc.tile_pool(name="nb", bufs=8) as nbp,
                tc.tile_pool(name="ob", bufs=6) as obp,
                tc.tile_pool(name="psC", bufs=7, space="PSUM") as psC,
            ):
                nbs = {}
                for grp in GROUPS:
                    for ti in grp:
                        t0 = ti * CH
                        lo = max(t0 - PAD, 0)
                        hi = min(t0 + CH + PAD, T)
                        dst_lo = lo - (t0 - PAD)      # 3 for first chunk else 0
                        dst_hi = dst_lo + (hi - lo)
                        xni = xnip.tile([128, NT, HALO], f32)
                        if dst_lo > 0:
                            nc.vector.memset(xni[:, :, 0:dst_lo], 0.0)
                        if dst_hi < HALO:
                            nc.vector.memset(xni[:, :, dst_hi:HALO], 0.0)
                        nc.sync.dma_start(
                            out=xni[:, :, dst_lo:dst_hi],
                            in_=xn_scr[:, :, lo:hi].rearrange("j p t -> p j t"))
                        qf = qfp.tile([128, NT, HALO], f32)
                        if apply_gamma:
                            for j in range(NT):
                                nc.scalar.activation(out=qf[:, j, :],
                                                     in_=xni[:, j, :],
                                                     func=ACTF.Copy,
                                                     scale=gs[j][:], bias=MAGIC)
                        else:
                            nc.scalar.activation(out=qf[:], in_=xni[:],
                                                 func=ACTF.Copy,
                                                 scale=gs[0][:], bias=MAGIC)
                        # two copies: even-k taps read nb, odd-k taps read nb1
                        # (shifted 1 elem) so every matmul rhs slice is 4-byte
                        # aligned (odd bf16 offsets fault the PE).
                        nb = nbp.tile([128, NT, HALO], bf16)
                        nc.vector.tensor_scalar_sub(nb[:], qf[:], MAGIC)
                        nb1 = nbp.tile([128, NT, HALO - 1], bf16)
                        nc.vector.tensor_copy(out=nb1[:], in_=nb[:, :, 1:HALO])
                        nbs[ti] = (nb, nb1)
                    for m in range(NT):
                        pcs = {}
                        for ti in grp:
                            pcs[ti] = psC.tile([128, CH], f32,
                                               name=f"pc{ti}", tag="pc")
                        nmm = NT * KS
                        idx = 0
                        for j in range(NT):
                            for k in range(KS):
                                w_sl = wqT_sl(k, j, m)
                                for ti in grp:
                                    if k % 2 == 0:
                                        rhs = nbs[ti][0][:, j, k:k + CH]
                                    else:
                                        rhs = nbs[ti][1][:, j, k - 1:k - 1 + CH]
                                    nc.tensor.matmul(
                                        pcs[ti][:], w_sl, rhs,
                                        start=(idx == 0),
                                        stop=(idx == nmm - 1))
                                idx += 1
                        for ti in grp:
                            ob = obp.tile([128, CH], f32)
                            nc.scalar.activation(out=ob[:], in_=pcs[ti][:],
                                                 func=ACTF.Copy, scale=osc[:])
                            nc.sync.dma_start(
                                out=out_ext[m * 128:(m + 1) * 128,
                                            ti * CH:ti * CH + CH],
                                in_=ob[:])

    nc.finalize()
    return nc


_NC_CACHE = {}


def _get_nc(apply_gamma: bool):
    key = (apply_gamma, os.environ.get("BITCONV_AG", "1"),
           os.environ.get("BITCONV_EXACT_RMS", "0"))
    if key not in _NC_CACHE:
        _NC_CACHE[key] = _build(apply_gamma)
    return _NC_CACHE[key]


def _run(x, weight, norm_weight, trace=False, tmpdir=None):
    x = np.ascontiguousarray(x, dtype=np.float32)
    weight = np.ascontiguousarray(weight, dtype=np.float32)
    norm_weight = np.ascontiguousarray(norm_weight, dtype=np.float32)
    assert x.shape == (N_CORES, C, T), x.shape
    assert weight.shape == (C, C, KS), weight.shape
    assert norm_weight.shape == (C,), norm_weight.shape
    # device wants lhsT layout [cin, k, cout] (pure layout permutation)
    weight = np.ascontiguousarray(weight.transpose(1, 2, 0))

    apply_gamma = not bool(np.all(norm_weight == np.float32(1.0)))
    nc = _get_nc(apply_gamma)
    in_maps = [
        {"x": x[i], "w": weight, "nw": norm_weight} for i in range(N_CORES)
    ]
    res = run_bass_kernel_spmd(nc, in_maps, list(range(N_CORES)),
                               trace=trace, tmpdir=tmpdir)
    out = np.stack([res.results[i]["out"] for i in range(N_CORES)], axis=0)
    return out, res.exec_time_ns


def kernel(x, weight, norm_weight):
    out, _ = _run(x, weight, norm_weight)
    return out



# revision 4
# speedup vs baseline: 1.0155x; 1.0155x over previous
"""BitConv1d Trainium2 kernel (8 NeuronCores, data-parallel over batch).

Reference semantics (per batch b):
    x_n   = rmsnorm_over_C(x) * gamma
    scale = max(|x_n|) over the WHOLE tensor (global -> AllGather + max)
    n     = round(clip(x_n / scale * 127, -128, 127))        (integers in [-127,127])
    w_s   = max(mean(|w|), 1e-4)
    w_q   = round(clip(w / w_s, -1, 1))                      (ternary)
    out   = conv1d(n, w_q, pad=3) * (scale/127) * w_s

Key insight: n is an integer |n|<=127 (exact in bf16) and w_q is ternary
(exact in bf16), so the conv is EXACT integer arithmetic on the PE in
bf16 with fp32 PSUM accumulation.  All rounding is done with the fp32
magic-number trick (+1.5*2^23, RNE) which matches jnp.round.

Structure (v2 — single-load scale pass + fused recompute conv pass):
  Phase A streams x once per 512-col chunk: sum_c x^2 via 4 accumulating
    all-ones fp16 matmuls; ACT-table rsqrt written into a persistent
    SBUF rms cache [128, T+6]; x*rms on DVE only to feed the chunk
    abs-max (gpsimd); NO x_n scratch writeback.  Weight quantization
    overlaps the stream (|w| sums on gpsimd).
  AllGather of the 1-scalar local max -> global scale; hidden behind
    phase B's x prefetch + rms-mul DVE work for the first 3 chunks.
  Phase B re-loads x with a 3-col halo, recomputes q = x*rms from the
    SBUF rms cache (no matmuls/rsqrt needed), quantizes via ACT
    (q*127/s + MAGIC, in place) + DVE (-MAGIC -> bf16), then runs the
    112 [128x128]@[128x512] conv matmuls per chunk back-to-back so the
    PE streams at the (power-throttled) roofline.  Even/odd-shifted
    bf16 copies keep every matmul rhs slice 4-byte aligned.
"""

import os
import sys
import types

import numpy as np


def _install_ntff_shim():
    """Make bass_utils' trace path work in containers lacking antenv.axon_hooks."""
    try:
        import antenv.axon_hooks  # noqa: F401
        return
    except ImportError:
        pass
    try:
        from trn_agent_boot.trn_boot import _ntff_profile_via_ctypes

        mod = types.ModuleType("antenv.axon_hooks")
        hook = _ntff_profile_via_ctypes("/opt/axon/libaxon_pjrt.so")
        mod.get_axon_ntff_profile_hook = lambda: hook
        mod.set_axon_ntff_profile_hook = lambda h: None
        sys.modules["antenv.axon_hooks"] = mod
        import antenv

        antenv.axon_hooks = mod
    except Exception:
        pass


_install_ntff_shim()

import concourse.bacc as bacc
import concourse.tile as tile
from concourse import mybir
from concourse.bass_utils import run_bass_kernel_spmd

f32 = mybir.dt.float32
bf16 = mybir.dt.bfloat16
f16 = mybir.dt.float16

N_CORES = 8
C = 512          # in/out channels
T = 8192         # sequence length
KS = 7           # kernel taps
PAD = 3
NT = 4           # channel tiles of 128
CH = 512         # T-chunk width
NCH = T // CH    # 16
EPS = 1e-6
QP = 127.0
MAGIC = 12582912.0        # 1.5 * 2**23 : fp32 round-to-nearest-int magic
W_ELEMS = C * C * KS      # 1835008
HALO = CH + 2 * PAD       # 518
PREFETCH = 3


def _build(apply_gamma: bool):
    Alu = mybir.AluOpType
    ACTF = mybir.ActivationFunctionType

    nc = bacc.Bacc("TRN2", target_bir_lowering=False, debug=False,
                   num_devices=N_CORES)

    x_ext = nc.dram_tensor("x", [C, T], f32, kind="ExternalInput")
    # host supplies weight transposed to [cin, k, cout] so quantized lhsT
    # tiles are contiguous slices (no on-chip transposes needed)
    w_ext = nc.dram_tensor("w", [C, KS, C], f32, kind="ExternalInput")
    nw_ext = nc.dram_tensor("nw", [C], f32, kind="ExternalInput")
    out_ext = nc.dram_tensor("out", [C, T], f32, kind="ExternalOutput")

    with tile.TileContext(nc) as tc:
        with (
            tc.tile_pool(name="consts", bufs=1) as consts,
            tc.tile_pool(name="wqt", bufs=1) as wqtp,
            tc.tile_pool(name="dram", bufs=1, space="DRAM") as dram,
        ):
            ones128 = consts.tile([128, 128], f32)
            nc.vector.memset(ones128[:], 1.0)
            ones_h = consts.tile([128, 128], f16)
            nc.vector.memset(ones_h[:], 1.0)
            eps_t = consts.tile([128, 1], f32)
            nc.vector.memset(eps_t[:], EPS)
            gamma = [consts.tile([128, 1], f32, name=f"gamma{j}") for j in range(NT)]
            for j in range(NT):
                nc.sync.dma_start(
                    out=gamma[j][:],
                    in_=nw_ext[j * 128:(j + 1) * 128].rearrange("(p o) -> p o", o=1),
                )
            # per-position rms cache, 3-col pad each side so halo slices
            # are always in range (pad cols multiply x=0 -> value irrelevant,
            # but must be finite)
            rms_all = consts.tile([128, T + 2 * PAD], f32)
            nc.vector.memset(rms_all[:, 0:PAD], 1.0)
            nc.vector.memset(rms_all[:, T + PAD:T + 2 * PAD], 1.0)
            mxbuf = consts.tile([128, NCH], f32)        # abs-max per chunk
            wsums = consts.tile([128, NT], f32)
            # post-collective scalars
            sc128 = consts.tile([128, 1], f32)      # global act scale
            s127 = consts.tile([128, 1], f32)       # 127/scale
            gs = [consts.tile([128, 1], f32, name=f"gs{j}") for j in range(NT)]
            ws128 = consts.tile([128, 1], f32)      # weight scale
            osc = consts.tile([128, 1], f32)        # w_s*scale/127
            mx1 = consts.tile([128, 1], f32)
            mxt = consts.tile([1, 128], f32)
            mxs = consts.tile([1, 1], f32)
            mxc = consts.tile([1, 1], f32)
            agt = consts.tile([1, N_CORES], f32)
            scs = consts.tile([1, 1], f32)
            sinv = consts.tile([128, 1], f32)

            # ternary weights, bf16, lhsT layout: tile j holds
            # [128 cin, (k, cout)] so slice (k, m) is contiguous
            wqTs = [wqtp.tile([128, KS * C], bf16, name=f"wqT{j}")
                    for j in range(NT)]

            def wqT_sl(k, j, m):
                return wqTs[j][:, k * C + m * 128: k * C + m * 128 + 128]

            ccin = dram.tile([1, 1], f32)
            ccag = dram.tile([N_CORES, 1], f32, addr_space="Shared")

            # ================= Phase A: scale pass =================
            with (
                tc.tile_pool(name="xin", bufs=4) as xinp,
                tc.tile_pool(name="sq", bufs=3) as sqp,
                tc.tile_pool(name="xna", bufs=3) as xnap,
                tc.tile_pool(name="wraw", bufs=4) as wrawp,
                tc.tile_pool(name="wsm", bufs=2) as wsmp,
                tc.tile_pool(name="psA", bufs=3, space="PSUM") as psA,
                tc.tile_pool(name="psW", bufs=1, space="PSUM") as psW,
            ):
                # ---- weight load + |w| sums (gpsimd; DMA overlaps) ----
                wraws = []
                for m in range(NT):
                    wraw = wrawp.tile([128, KS * C], f32)
                    nc.sync.dma_start(
                        out=wraw[:],
                        in_=w_ext[m * 128:(m + 1) * 128, :, :].rearrange(
                            "p k c -> p (k c)"))
                    wraws.append(wraw)
                    t56 = wsmp.tile([128, 56], f32)
                    nc.vector.tensor_reduce(
                        out=t56[:],
                        in_=wraw[:].rearrange("p (a b) -> p a b", b=64),
                        axis=mybir.AxisListType.X, op=Alu.add,
                        apply_absolute_value=True)
                    nc.vector.tensor_reduce(
                        out=wsums[:, m:m + 1], in_=t56[:],
                        axis=mybir.AxisListType.X, op=Alu.add)

                def w_scale_setup():
                    wtot = wsmp.tile([128, 1], f32)
                    nc.vector.tensor_reduce(out=wtot[:], in_=wsums[:],
                                            axis=mybir.AxisListType.X,
                                            op=Alu.add)
                    pws = psW.tile([128, 1], f32)
                    nc.tensor.matmul(pws[:], ones128[:], wtot[:],
                                     start=True, stop=True)
                    wmean = wsmp.tile([128, 1], f32)
                    nc.scalar.activation(out=wmean[:], in_=pws[:],
                                         func=ACTF.Copy, scale=1.0 / W_ELEMS)
                    nc.vector.tensor_scalar_max(ws128[:], wmean[:], 1e-4)
                    winv = wsmp.tile([128, 1], f32)
                    nc.vector.reciprocal(winv[:], ws128[:])
                    for m in range(NT):
                        # in-place: wraw <- round(w/ws)+MAGIC, clip to MAGIC+-1
                        nc.scalar.activation(out=wraws[m][:], in_=wraws[m][:],
                                             func=ACTF.Copy, scale=winv[:],
                                             bias=MAGIC)
                        nc.gpsimd.tensor_scalar(out=wraws[m][:],
                                                in0=wraws[m][:],
                                                scalar1=MAGIC + 1.0,
                                                scalar2=MAGIC - 1.0,
                                                op0=Alu.min, op1=Alu.max)

                def w_convert(j, half):
                    # -MAGIC and cast: ternary values, exact in bf16
                    h = (KS * C) // 2
                    nc.vector.tensor_scalar_sub(
                        wqTs[j][:, half * h:(half + 1) * h],
                        wraws[j][:, half * h:(half + 1) * h],
                        MAGIC)

                # ---- stream 16 chunks: rms into cache + local abs-max ----
                for ti in range(NCH):
                    if ti == 1:
                        w_scale_setup()
                    t0 = ti * CH
                    ps = psA.tile([128, CH], f32)
                    # one 3-D DMA brings all four channel tiles of the chunk
                    xt = xinp.tile([128, NT, CH], f32)
                    nc.sync.dma_start(
                        out=xt[:],
                        in_=x_ext[:, t0:t0 + CH].rearrange(
                            "(j p) t -> p j t", p=128))
                    sq = sqp.tile([128, NT, CH], f16)
                    nc.scalar.square(sq[:], xt[:])
                    for j in range(NT):
                        # accumulate sum_c x^2 on the PE; all-ones lhsT also
                        # broadcasts the result to every partition
                        nc.tensor.matmul(ps[:], ones_h[:], sq[:, j, :],
                                         start=(j == 0), stop=(j == NT - 1))
                    if 2 <= ti <= 5:
                        w_convert(ti - 2, 0)
                        w_convert(ti - 2, 1)
                    # table rsqrt (max rel err ~4e-5) straight into the cache
                    nc.scalar.activation(
                        out=rms_all[:, PAD + t0:PAD + t0 + CH], in_=ps[:],
                        func=ACTF.Abs_reciprocal_sqrt,
                        bias=eps_t[:], scale=1.0 / C)
                    xna = xnap.tile([128, NT, CH], f32)
                    for j in range(NT):
                        xsl = xna[:, j, :]
                        if apply_gamma:
                            nc.vector.tensor_scalar_mul(xsl, xt[:, j, :],
                                                        gamma[j][:])
                            nc.vector.tensor_mul(
                                xsl, xsl, rms_all[:, PAD + t0:PAD + t0 + CH])
                        else:
                            nc.vector.tensor_mul(
                                xsl, xt[:, j, :],
                                rms_all[:, PAD + t0:PAD + t0 + CH])
                    # one abs-max reduce per chunk covers all four tiles
                    nc.vector.tensor_reduce(
                        out=mxbuf[:, ti:ti + 1], in_=xna[:],
                        axis=mybir.AxisListType.XY, op=Alu.max,
                        apply_absolute_value=True)

                # ---- local max tree ----
                nc.vector.tensor_reduce(out=mx1[:], in_=mxbuf[:],
                                        axis=mybir.AxisListType.X, op=Alu.max)
                nc.sync.dma_start(out=mxt[:], in_=mx1[:])
                nc.vector.tensor_reduce(out=mxs[:], in_=mxt[:],
                                        axis=mybir.AxisListType.X, op=Alu.max)
                nc.vector.tensor_scalar_max(mxc[:], mxs[:], 1e-5)
                nc.gpsimd.dma_start(out=ccin[:], in_=mxc[:])

            # ============ Phase B: quantize + conv matmuls ============
            with (
                tc.tile_pool(name="xh", bufs=4) as xhp,
                tc.tile_pool(name="qf", bufs=4) as qfp,
                tc.tile_pool(name="nb", bufs=6) as nbp,
                tc.tile_pool(name="ob", bufs=6) as obp,
                tc.tile_pool(name="psC", bufs=6, space="PSUM") as psC,
            ):
                nc.gpsimd.collective_compute(
                    "AllGather", Alu.bypass,
                    replica_groups=[list(range(N_CORES))],
                    ins=[ccin.opt()], outs=[ccag.opt()],
                )

                qtiles = {}

                def prefetch(ti):
                    # x reload with halo + rms-mul; none of this needs the
                    # global scale, so it overlaps the collective
                    t0 = ti * CH
                    lo = max(t0 - PAD, 0)
                    hi = min(t0 + CH + PAD, T)
                    dst_lo = lo - (t0 - PAD)      # 3 for first chunk else 0
                    dst_hi = dst_lo + (hi - lo)
                    xh = xhp.tile([128, NT, HALO], f32)
                    if dst_lo > 0:
                        nc.vector.memset(xh[:, :, 0:dst_lo], 0.0)
                    if dst_hi < HALO:
                        nc.vector.memset(xh[:, :, dst_hi:HALO], 0.0)
                    nc.sync.dma_start(
                        out=xh[:, :, dst_lo:dst_hi],
                        in_=x_ext[:, lo:hi].rearrange("(j p) t -> p j t",
                                                      p=128))
                    q = qfp.tile([128, NT, HALO], f32)
                    for j in range(NT):
                        nc.vector.tensor_mul(q[:, j, :], xh[:, j, :],
                                             rms_all[:, t0:t0 + HALO])
                    qtiles[ti] = q

                for ti in range(PREFETCH):
                    prefetch(ti)

                # ---- post-collective scalar setup ----
                nc.gpsimd.dma_start(out=agt[:],
                                    in_=ccag[:].rearrange("r o -> o r"))
                nc.vector.tensor_reduce(out=scs[:], in_=agt[:],
                                        axis=mybir.AxisListType.X, op=Alu.max)
                nc.gpsimd.partition_broadcast(sc128[:], scs[:])
                nc.vector.reciprocal(sinv[:], sc128[:])
                nc.vector.tensor_scalar_mul(s127[:], sinv[:], QP)
                if apply_gamma:
                    for j in range(NT):
                        nc.vector.tensor_mul(gs[j][:], gamma[j][:], s127[:])
                nc.vector.tensor_mul(osc[:], ws128[:], sc128[:])
                nc.vector.tensor_scalar_mul(osc[:], osc[:], 1.0 / QP)

                for ti in range(NCH):
                    q = qtiles.pop(ti)
                    # quantize: q <- q*127/s + MAGIC (in place), then
                    # subtract MAGIC -> bf16 integers
                    if apply_gamma:
                        for j in range(NT):
                            nc.scalar.activation(out=q[:, j, :],
                                                 in_=q[:, j, :],
                                                 func=ACTF.Copy,
                                                 scale=gs[j][:], bias=MAGIC)
                    else:
                        nc.scalar.activation(out=q[:], in_=q[:],
                                             func=ACTF.Copy,
                                             scale=s127[:], bias=MAGIC)
                    # two copies: even-k taps read nb, odd-k taps read nb1
                    # (shifted 1 elem) so every matmul rhs slice is 4-byte
                    # aligned (odd bf16 offsets fault the PE).
                    nb = nbp.tile([128, NT, HALO], bf16)
                    nc.vector.tensor_scalar_sub(nb[:], q[:], MAGIC)
                    nb1 = nbp.tile([128, NT, HALO - 1], bf16)
                    nc.vector.tensor_copy(out=nb1[:], in_=nb[:, :, 1:HALO])
                    if ti + PREFETCH < NCH:
                        prefetch(ti + PREFETCH)
                    for m in range(NT):
                        pc = psC.tile([128, CH], f32)
                        idx = 0
                        for j in range(NT):
                            for k in range(KS):
                                if k % 2 == 0:
                                    rhs = nb[:, j, k:k + CH]
                                else:
                                    rhs = nb1[:, j, k - 1:k - 1 + CH]
                                nc.tensor.matmul(
                                    pc[:], wqT_sl(k, j, m), rhs,
                                    start=(idx == 0), stop=(idx == NT * KS - 1))
                                idx += 1
                        ob = obp.tile([128, CH], f32)
                        nc.scalar.activation(out=ob[:], in_=pc[:],
                                             func=ACTF.Copy, scale=osc[:])
                        nc.sync.dma_start(
                            out=out_ext[m * 128:(m + 1) * 128,
                                        ti * CH:ti * CH + CH],
                            in_=ob[:])

    nc.finalize()
    return nc


_NC_CACHE = {}


def _get_nc(apply_gamma: bool):
    if apply_gamma not in _NC_CACHE:
        _NC_CACHE[apply_gamma] = _build(apply_gamma)
    return _NC_CACHE[apply_gamma]


def _run(x, weight, norm_weight, trace=False, tmpdir=None):
    x = np.ascontiguousarray(x, dtype=np.float32)
    weight = np.ascontiguousarray(weight, dtype=np.float32)
    norm_weight = np.ascontiguousarray(norm_weight, dtype=np.float32)
    assert x.shape == (N_CORES, C, T), x.shape
    assert weight.shape == (C, C, KS), weight.shape
    assert norm_weight.shape == (C,), norm_weight.shape
    # device wants lhsT layout [cin, k, cout] (pure layout permutation)
    weight = np.ascontiguousarray(weight.transpose(1, 2, 0))

    apply_gamma = not bool(np.all(norm_weight == np.float32(1.0)))
    nc = _get_nc(apply_gamma)
    in_maps = [
        {"x": x[i], "w": weight, "nw": norm_weight} for i in range(N_CORES)
    ]
    res = run_bass_kernel_spmd(nc, in_maps, list(range(N_CORES)),
                               trace=trace, tmpdir=tmpdir)
    out = np.stack([res.results[i]["out"] for i in range(N_CORES)], axis=0)
    return out, res.exec_time_ns


def kernel(x, weight, norm_weight):
    out, _ = _run(x, weight, norm_weight)
    return out
